# revision 1
# baseline (speedup 1.0000x reference)
"""MI-estimator loss kernel for 8 Trainium2 NeuronCores.

Math (reference):
    mu     = relu(x @ w1 + b1) @ w2 + b2
    logvar = tanh(relu(x @ v1 + c1) @ v2 + c2)
    ivar   = exp(-logvar)
    loss   = mean_i sum_d [pos - neg]
           = -0.5/N * sum_{i,d} ivar*(y^2 - 2*mu*y + 2*mu*ybar_d - y2bar_d)

The loss is linear in the global stats ybar/y2bar, so each core only needs
local reductions over its rows i:
    B[d] = sum_i ivar,  E[d] = sum_i mu*ivar,
    A    = sum_{i,d} ivar*y^2,  C = sum_{i,d} mu*ivar*y
and the host combines:
    loss = -0.5/N * (A - 2C + sum_d (2*E[d]*ybar[d] - B[d]*y2bar[d]))

Sharding: data-parallel over N=8192 rows -> 1024 rows/core; weights broadcast.
Device layout: features on partitions. Host passes x.T per shard (256,1024);
the device returns ivar and mi = (mu+b2)*ivar as (64,1024) tensors and the
host does the tiny reductions against y (the emb_y shards never go to the
device at all).
"""

import sys

import numpy as np

try:
    import concourse.bass  # noqa: F401
except ImportError:
    for p in ("/opt/trn_rl_repo", "/root/.axon_site/_ro/trn_rl_repo"):
        if p not in sys.path:
            sys.path.insert(0, p)

N, DX, DY, H = 8192, 256, 64, 256
NCORES = 8
NLOC = N // NCORES  # 1024 rows per core
NH = NLOC // 2  # 512, one PSUM bank of fp32
WCOLS = 2 * H + 2 * DY + 6  # packed weights+biases columns
W1C = 2 * H + 6  # w1 halves + bias columns (first DMA chunk)

_CACHE = {}


def _build_nc():
    import concourse.bass as bass
    import concourse.mybir as mybir
    import concourse.tile as tile
    from concourse import bacc
    from concourse.bass import _add_dep_helper

    f32 = mybir.dt.float32
    f32r = mybir.dt.float32r
    AF = mybir.ActivationFunctionType
    ALU = mybir.AluOpType

    nc = bacc.Bacc(
        trn_type="TRN2",
        target_bir_lowering=False,
        debug=False,
        num_devices=NCORES,
    )

    xT = nc.dram_tensor("xT", (DX, NLOC), f32r, kind="ExternalInput").ap()
    # all weights + biases in one tensor, split into a w1+bias chunk and a
    # w2 chunk per contraction half: cols 0:256 mu_w1 | 256:512 lv_w1 |
    # 512:518 bias columns (0,1 mu_b1 halves; 2,3 lv_b1 halves; 4 mu_b2;
    # 5 lv_b2 -- valid in rows 0:128) | 518:582 mu_w2 | 582:646 lv_w2
    wpk = nc.dram_tensor("wpk", (DX, WCOLS), f32r, kind="ExternalInput").ap()
    oiv = nc.dram_tensor("oiv", (DY, NLOC), f32, kind="ExternalOutput").ap()
    omi = nc.dram_tensor("omi", (DY, NLOC), f32, kind="ExternalOutput").ap()

    with tile.TileContext(nc) as tc:
        with (
            tc.tile_pool(name="const", bufs=1) as const,
            tc.tile_pool(name="xp", bufs=1) as xp,
            tc.tile_pool(name="hp", bufs=1) as hp,
            tc.tile_pool(name="wk", bufs=1) as wk,
            tc.tile_pool(name="psp", bufs=1, space="PSUM") as psp,
        ):
            # ---- loads, in PE consumption order ------------------------
            w1_sb = [None, None]
            x_sb = {}

            def load_w1(k):
                t = const.tile([128, W1C], f32r, tag=f"w1{k}")
                nc.sync.dma_start(out=t, in_=wpk[k * 128 : (k + 1) * 128, 0:W1C])
                w1_sb[k] = t

            def load_x(k, h):
                t = xp.tile([128, NH], f32r, tag=f"x{k}{h}")
                nc.sync.dma_start(
                    out=t,
                    in_=xT[k * 128 : (k + 1) * 128, h * NH : (h + 1) * NH],
                )
                x_sb[(k, h)] = t

            load_w1(0)
            load_x(0, 0)
            load_x(0, 1)
            load_w1(1)
            load_x(1, 0)
            load_x(1, 1)
            w2_sb = []
            for k in range(2):
                t = const.tile([128, 2 * DY], f32r, tag=f"w2{k}")
                nc.sync.dma_start(
                    out=t, in_=wpk[k * 128 : (k + 1) * 128, W1C:WCOLS]
                )
                w2_sb.append(t)
            def w1_ap(head, k, m):
                off = 0 if head == "mu" else H
                return w1_sb[k][:, off + m * 128 : off + (m + 1) * 128]

            def w2_ap(head, k):
                off = 0 if head == "mu" else DY
                return w2_sb[k][:, off : off + DY]

            def bias_ap(j, p=128):
                return w1_sb[0][0:p, 2 * H + j][:, None].bitcast(f32)

            # One PSUM tensor spanning all 8 banks, sub-ranged manually.
            # Within one tensor, PE-write-after-PE-write needs no semaphore,
            # so bank reuse (L2 outputs overwrite L1 banks) costs only the
            # WAR wait against the relu that read them -- the same
            # instruction the L2 matmul already waits on for its rhs. This
            # matters because fp32r matmuls (self-loading weights, S3_LW
            # encoding) have a single sync-wait slot.
            # Bank map (bank b = cols [512b, 512(b+1))):
            #   b0,b1: L1 lv m0 (then L2 lv rows 0:64); b2,b3: L1 lv m1
            #   b4,b5: L1 mu m0 (then L2 mu rows 0:64); b6,b7: L1 mu m1
            ps_all = psp.tile([128, 8 * NH], f32, tag="ps")

            # Pin PE issue order with no-sync edges: the scheduler otherwise
            # reorders matmuls and delays the lv head, whose tail
            # (tanh->exp->products) is the long serial chain.
            _prev_mm = [None]

            def mm(out_ap, lhsT, rhs, start, stop):
                m = nc.tensor.matmul(out_ap, lhsT=lhsT, rhs=rhs, start=start,
                                     stop=stop)
                if _prev_mm[0] is not None:
                    _add_dep_helper(m.ins, _prev_mm[0].ins, sync=False,
                                    reason="pin PE order")
                _prev_mm[0] = m
                return m

            # PE warmup: the HAM clock gate holds the PE at 1.2 GHz until it
            # has been busy ~3.4us. Run garbage matmuls while the DMAs load
            # so the real matmuls run at 2.4 GHz. Results land in bank 0,
            # which the first real accumulation group clears (start=True).
            warm = const.tile([128, NH], f32, tag="warm")
            nc.gpsimd.memset(warm, 0.0)
            warm_r = warm.bitcast(f32r)
            for _ in range(6):
                mm(ps_all[:, 0:NH], warm_r[:, 0:128], warm_r, True, True)

            # ---- two MLP heads (lv first: its tail is the long chain) ----
            # L1 runs as two k-passes: all k0 matmuls (start=True) stream as
            # soon as x0 lands while x1 is still in flight, then the k1 pass
            # accumulates (stop=True). Groups complete in order, so each
            # relu fires right after its group's k1 matmul.
            # L2: rows 0:64, h-halves side by side in the free dim
            l1_base = {("lv", 0): 0, ("lv", 1): 2 * NH,
                       ("mu", 0): 4 * NH, ("mu", 1): 6 * NH}
            l2_base = {"lv": 0, "mu": 4 * NH}
            hT = {}
            GROUPS = [("lv", 0), ("lv", 1), ("mu", 0), ("mu", 1)]

            # relu halves, balanced so ACT frees up for the tanh/exp chain
            RELU_ENG = {("lv", 0, 0): "act", ("lv", 0, 1): "act",
                        ("lv", 1, 0): "dve", ("lv", 1, 1): "dve",
                        ("mu", 0, 0): "dve", ("mu", 0, 1): "act",
                        ("mu", 1, 0): "dve", ("mu", 1, 1): "dve"}

            relu_insts = {}

            def relu_half(head, m, h):
                base = l1_base[(head, m)]
                ht = hT[(head, m)]
                bias_col = bias_ap((0 if head == "mu" else 2) + m)
                sl = slice(h * NH, (h + 1) * NH)
                if RELU_ENG[(head, m, h)] == "act":
                    relu_insts[(head, m, h)] = nc.scalar.activation(
                        out=ht[:, sl],
                        in_=ps_all[:, base + h * NH : base + (h + 1) * NH],
                        func=AF.Relu,
                        bias=bias_col,
                    )
                else:
                    relu_insts[(head, m, h)] = nc.vector.tensor_scalar(
                        out=ht[:, sl],
                        in0=ps_all[:, base + h * NH : base + (h + 1) * NH],
                        scalar1=bias_col,
                        scalar2=0.0,
                        op0=ALU.add,
                        op1=ALU.max,
                    )

            def l1_mm(head, m, k, h):
                base = l1_base[(head, m)]
                mm(
                    ps_all[:, base + h * NH : base + (h + 1) * NH],
                    w1_ap(head, k, m),
                    x_sb[(k, h)],
                    k == 0,
                    k == 1,
                )

            def l2_half(head, h):
                base2 = l2_base[head]
                for k in range(2):
                    mm(
                        ps_all[0:DY, base2 + h * NH : base2 + (h + 1) * NH],
                        w2_ap(head, k),
                        hT[(head, k)][:, h * NH : (h + 1) * NH],
                        k == 0,
                        k == 1,
                    )

            for head, m in GROUPS:
                ht = hp.tile([128, NLOC], f32r, tag=f"hT{head}{m}")
                hT[(head, m)] = ht

            # passes (k0,h0) (k0,h1) (k1,h0): stream behind the x DMAs
            for k, h in ((0, 0), (0, 1), (1, 0)):
                for head, m in GROUPS:
                    l1_mm(head, m, k, h)
            # h0 halves of every group are now complete
            for head, m in GROUPS:
                relu_half(head, m, 0)
            # last pass, lv groups first, with L2-lv-h0 slotted in between
            # so the tanh->exp->mi chain starts as early as possible
            l1_mm("lv", 0, 1, 1)
            l1_mm("lv", 1, 1, 1)
            relu_half("lv", 0, 1)
            relu_half("lv", 1, 1)
            l2_half("lv", 0)
            l1_mm("mu", 0, 1, 1)
            l1_mm("mu", 1, 1, 1)
            relu_half("mu", 0, 1)
            relu_half("mu", 1, 1)
            l2_half("lv", 1)
            l2_half("mu", 0)
            l2_half("mu", 1)
            lv_ps = ps_all[0:DY, l2_base["lv"] : l2_base["lv"] + NLOC]
            mu_ps = ps_all[0:DY, l2_base["mu"] : l2_base["mu"] + NLOC]

            # ---- tail, pipelined in n-halves --------------------------
            # Only ivar and mi are computed on-device; the cheap reductions
            # (B, E, A, C) happen on the host from the shipped tensors, so
            # the DVE tail is just two ops and the idle DMA engines carry
            # the results out.
            lg = wk.tile([DY, NLOC], f32, tag="lg")
            iv = wk.tile([DY, NLOC], f32, tag="iv")
            mi = wk.tile([DY, NLOC], f32, tag="mi")

            _prev_act = [None]

            def chain_act(ins):
                if _prev_act[0] is not None:
                    _add_dep_helper(ins.ins, _prev_act[0].ins, sync=False,
                                    reason="pin ACT order")
                _prev_act[0] = ins

            for h in range(2):
                sl = slice(h * NH, (h + 1) * NH)
                t = nc.scalar.activation(
                    out=lg[:, sl], in_=lv_ps[:, sl], func=AF.Tanh,
                    bias=bias_ap(5, DY),
                )
                chain_act(t)
                e = nc.scalar.activation(
                    out=iv[:, sl], in_=lg[:, sl], func=AF.Exp, scale=-1.0,
                )
                chain_act(e)
                nc.sync.dma_start(out=oiv[:, sl], in_=iv[:, sl])
                # mi = (mu_psum + b2) * ivar
                nc.vector.scalar_tensor_tensor(
                    out=mi[:, sl],
                    in0=mu_ps[:, sl],
                    scalar=bias_ap(4, DY),
                    in1=iv[:, sl],
                    op0=ALU.add,
                    op1=ALU.mult,
                )
                nc.sync.dma_start(out=omi[:, sl], in_=mi[:, sl])

    nc.compile()
    return nc


def _get_nc():
    if "nc" not in _CACHE:
        _CACHE["nc"] = _build_nc()
    return _CACHE["nc"]


def _make_in_maps(inputs):
    # convert everything to numpy up front: slicing jax arrays here could
    # otherwise dispatch to the (axon) device backend
    inputs = {k: np.asarray(v) for k, v in inputs.items()}
    emb_x = np.asarray(inputs["emb_x"], dtype=np.float32)
    emb_y = np.asarray(inputs["emb_y"], dtype=np.float32)

    bias = np.zeros((DX, 6), dtype=np.float32)
    bias[:128, 0] = np.asarray(inputs["mu_b1"][:128], np.float32)
    bias[:128, 1] = np.asarray(inputs["mu_b1"][128:], np.float32)
    bias[:128, 2] = np.asarray(inputs["lv_b1"][:128], np.float32)
    bias[:128, 3] = np.asarray(inputs["lv_b1"][128:], np.float32)
    bias[:128, 4] = np.tile(np.asarray(inputs["mu_b2"], np.float32), 2)
    bias[:128, 5] = np.tile(np.asarray(inputs["lv_b2"], np.float32), 2)

    wpk = np.concatenate(
        [
            np.asarray(inputs["mu_w1"], np.float32),
            np.asarray(inputs["lv_w1"], np.float32),
            bias,
            np.asarray(inputs["mu_w2"], np.float32),
            np.asarray(inputs["lv_w2"], np.float32),
        ],
        axis=1,
    )  # (256, 646)

    shared = {"wpk": np.ascontiguousarray(wpk)}

    in_maps = []
    for c in range(NCORES):
        rows = slice(c * NLOC, (c + 1) * NLOC)
        xsh = emb_x[rows]  # (1024, 256)
        in_maps.append(
            {
                "xT": np.ascontiguousarray(xsh.T),
                **shared,
            }
        )
    return in_maps


def kernel(emb_x, emb_y, mu_w1, mu_b1, mu_w2, mu_b2, lv_w1, lv_b1, lv_w2, lv_b2):
    from concourse.bass_utils import run_bass_kernel_spmd

    emb_y = np.asarray(emb_y, dtype=np.float32)
    in_maps = _make_in_maps(
        {
            "emb_x": emb_x,
            "emb_y": emb_y,
            "mu_w1": mu_w1,
            "mu_b1": mu_b1,
            "mu_w2": mu_w2,
            "mu_b2": mu_b2,
            "lv_w1": lv_w1,
            "lv_b1": lv_b1,
            "lv_w2": lv_w2,
            "lv_b2": lv_b2,
        }
    )

    nc = _get_nc()
    res = run_bass_kernel_spmd(nc, in_maps, list(range(NCORES)))

    B = np.zeros(DY)
    E = np.zeros(DY)
    A = 0.0
    C = 0.0
    for c in range(NCORES):
        yT = emb_y[c * NLOC : (c + 1) * NLOC].T.astype(np.float64)  # (64,1024)
        ivc = res.results[c]["oiv"].astype(np.float64)
        mic = res.results[c]["omi"].astype(np.float64)
        B += ivc.sum(axis=1)
        E += mic.sum(axis=1)
        A += (ivc * yT**2).sum()
        C += (mic * yT).sum()

    y64 = emb_y.astype(np.float64)
    ybar = y64.mean(axis=0)
    y2bar = (y64**2).mean(axis=0)

    total = A - 2.0 * C + (2.0 * E * ybar - B * y2bar).sum()
    loss = -0.5 / N * total
    return np.float32(loss)



# revision 9
# speedup vs baseline: 1.1467x; 1.1467x over previous
"""MI-estimator loss kernel for 8 Trainium2 NeuronCores.

Math (reference):
    mu     = relu(x @ w1 + b1) @ w2 + b2
    logvar = tanh(relu(x @ v1 + c1) @ v2 + c2)
    ivar   = exp(-logvar)
    loss   = -0.5/N * sum_{i,d} ivar*(y^2 - 2*mu*y + 2*mu*ybar_d - y2bar_d)

The device computes only the two MLP heads (up to the raw L2 PSUM, no
output biases) and ships raw mu and raw logvar back; the host applies
b2, tanh, exp and all reductions against emb_y in float64. emb_y never
goes to the device, and the serial ACT tanh->exp tail is gone entirely.

Sharding: data-parallel over N=8192 rows -> 1024 rows/core; weights
broadcast. All matmul operands are bf16 (halves DMA bytes; the PE rate
is the same as f32r here); PSUM stays fp32.

Layout: features on partitions. All bf16 inputs live in ONE packed
DRAM tensor loaded as 4 column-range DMAs (the DMA front is HWDGE-
bound at ~625ns/DMA, so fewer+bigger beats many small). L2 outputs are
written with the two n-halves STACKED on PSUM partitions (h0 ->
partitions 0:64, h1 -> 64:128), so each head's result is one (128,512)
tile: one copy op + one DMA out.

Packed tensor pk (128, 3328) bf16, columns:
    0:256      lv_w1[0:128]   (k0)        \  chunk c1 (with x0h0): what
    256:768    xT[0:128, 0:512]   (x0h0)  /  the first matmuls need
    768:1024   mu_w1[0:128]   (k0)
    1024:1536  xT[0:128, 512:1024] (x0h1)
    1536:2560  xT[128:256, :]     (x1)
    2560:2816  mu_w1[128:256] (k1)
    2816:3072  lv_w1[128:256] (k1)
    3072:3328  w2 pack: mu_w2[0:128] | lv_w2[0:128] | mu_w2[128:] | lv_w2[128:]
"""

import sys

import numpy as np

try:
    import concourse.bass  # noqa: F401
except ImportError:
    for p in ("/opt/trn_rl_repo", "/root/.axon_site/_ro/trn_rl_repo"):
        if p not in sys.path:
            sys.path.insert(0, p)

N, DX, DY, H = 8192, 256, 64, 256
NCORES = 8
NLOC = N // NCORES  # 1024 rows per core
NH = NLOC // 2  # 512, one PSUM bank of fp32

PK_C = 3328

_CACHE = {}


def _build_nc():
    import concourse.bass as bass
    import concourse.mybir as mybir
    import concourse.tile as tile
    from concourse import bacc
    from concourse.bass import _add_dep_helper

    f32 = mybir.dt.float32
    bf16 = mybir.dt.bfloat16
    AF = mybir.ActivationFunctionType
    ALU = mybir.AluOpType

    nc = bacc.Bacc(
        trn_type="TRN2",
        target_bir_lowering=False,
        debug=False,
        num_devices=NCORES,
    )

    pk = nc.dram_tensor("pk", (128, PK_C), bf16, kind="ExternalInput").ap()
    # bias (128, 4) f32: mu_b1 half0, mu_b1 half1, lv_b1 half0, lv_b1 half1
    bias = nc.dram_tensor("bias", (128, 4), f32, kind="ExternalInput").ap()
    # outputs: stacked (128, 512) f32: partitions 0:64 = n-half0 (rows d),
    # partitions 64:128 = n-half1
    omu = nc.dram_tensor("omu", (128, NH), f32, kind="ExternalOutput").ap()
    olv = nc.dram_tensor("olv", (128, NH), f32, kind="ExternalOutput").ap()

    with tile.TileContext(nc) as tc:
        with (
            tc.tile_pool(name="const", bufs=1) as const,
            tc.tile_pool(name="wk", bufs=1) as wk,
            tc.tile_pool(name="psp", bufs=1, space="PSUM") as psp,
        ):
            # ---- loads: 4 chunks of pk, in PE consumption order ---------
            pk_sb = const.tile([128, PK_C], bf16, tag="pk")
            nc.sync.dma_start(out=pk_sb[:, 0:768], in_=pk[:, 0:768])
            nc.sync.dma_start(out=pk_sb[:, 768:1536], in_=pk[:, 768:1536])
            nc.sync.dma_start(out=pk_sb[:, 1536:2560], in_=pk[:, 1536:2560])
            bias_sb = const.tile([128, 4], f32, tag="bias")
            nc.sync.dma_start(out=bias_sb, in_=bias)
            nc.sync.dma_start(out=pk_sb[:, 2560:PK_C], in_=pk[:, 2560:PK_C])

            W1_OFF = {("lv", 0): 0, ("mu", 0): 768,
                      ("mu", 1): 2560, ("lv", 1): 2816}
            X_OFF = {(0, 0): 256, (0, 1): 1024, (1, 0): 1536, (1, 1): 2048}

            def w1_ap(head, k, m):
                off = W1_OFF[(head, k)] + m * 128
                return pk_sb[:, off : off + 128]

            def x_ap(k, h):
                off = X_OFF[(k, h)]
                return pk_sb[:, off : off + NH]

            def w2_ap(head, k):
                off = 3072 + (2 * k + (0 if head == "mu" else 1)) * DY
                return pk_sb[:, off : off + DY]

            def bias_ap(j, p=128):
                return bias_sb[0:p, j][:, None]

            # One PSUM tensor spanning all 8 banks, sub-ranged manually.
            # Bank map (bank b = cols [512b, 512(b+1))):
            #   b0,b1: L1 lv m0 h0/h1 (b0 then takes L2 lv, stacked 128p)
            #   b2,b3: L1 lv m1; b4,b5: L1 mu m0 (b4 takes L2 mu);
            #   b6,b7: L1 mu m1
            ps_all = psp.tile([128, 8 * NH], f32, tag="ps")

            # Pin PE issue order with no-sync edges (the scheduler otherwise
            # reorders matmuls).
            _prev_mm = [None]

            def mm(out_ap, lhsT, rhs, start, stop):
                m = nc.tensor.matmul(out_ap, lhsT=lhsT, rhs=rhs, start=start,
                                     stop=stop)
                if _prev_mm[0] is not None:
                    _add_dep_helper(m.ins, _prev_mm[0].ins, sync=False,
                                    reason="pin PE order")
                _prev_mm[0] = m
                return m

            # PE warmup: the clock gate holds the PE below 2.4 GHz until it
            # has been busy ~3us; run garbage matmuls while the DMAs load.
            # Results land in bank 0, cleared by the first real accumulation
            # group (start=True).
            warm = const.tile([128, NH], f32, tag="warm")
            nc.vector.memset(warm, 0.0)
            warm_r = warm.bitcast(bf16)
            for _ in range(6):
                mm(ps_all[:, 0:NH], warm_r[:, 0:128], warm_r[:, 0:NH], True,
                   True)

            l1_base = {("lv", 0): 0, ("lv", 1): 2 * NH,
                       ("mu", 0): 4 * NH, ("mu", 1): 6 * NH}
            GROUPS = [("lv", 0), ("lv", 1), ("mu", 0), ("mu", 1)]

            hT = {}
            for head, m in GROUPS:
                ht = wk.tile([128, NLOC], bf16, tag=f"hT{head}{m}")
                hT[(head, m)] = ht

            # relu engine map: gpsimd cannot read PSUM, so ACT and DVE split
            # the eight halves 4/4.
            RELU_ENG = {("lv", 0, 0): "act", ("lv", 0, 1): "act",
                        ("lv", 1, 0): "dve", ("lv", 1, 1): "dve",
                        ("mu", 0, 0): "act", ("mu", 0, 1): "act",
                        ("mu", 1, 0): "dve", ("mu", 1, 1): "dve"}

            _prev_eng = {"act": [None], "dve": [None], "gp": [None]}

            def chain(eng, ins):
                slot = _prev_eng[eng]
                if slot[0] is not None:
                    _add_dep_helper(ins.ins, slot[0].ins, sync=False,
                                    reason=f"pin {eng} order")
                slot[0] = ins

            def relu_half(head, m, h):
                base = l1_base[(head, m)]
                ht = hT[(head, m)]
                bias_col = bias_ap((0 if head == "mu" else 2) + m)
                sl = slice(h * NH, (h + 1) * NH)
                ps = ps_all[:, base + h * NH : base + (h + 1) * NH]
                eng = RELU_ENG[(head, m, h)]
                if eng == "act":
                    i = nc.scalar.activation(out=ht[:, sl], in_=ps,
                                             func=AF.Relu, bias=bias_col)
                else:
                    i = nc.vector.tensor_scalar(
                        out=ht[:, sl], in0=ps, scalar1=bias_col, scalar2=0.0,
                        op0=ALU.add, op1=ALU.max)
                chain(eng, i)

            def l1_mm(head, m, k, h):
                base = l1_base[(head, m)]
                mm(ps_all[:, base + h * NH : base + (h + 1) * NH],
                   w1_ap(head, k, m), x_ap(k, h), k == 0, k == 1)

            # L2 outputs, n-halves stacked on partitions: h0 -> rows 0:64,
            # h1 -> rows 64:128. lv -> bank 0, mu -> bank 4.
            L2_BANK = {"lv": 0, "mu": 4 * NH}

            def l2_half(head, h):
                base = L2_BANK[head]
                out = ps_all[h * DY : (h + 1) * DY, base : base + NH]
                for k in range(2):
                    mm(out, w2_ap(head, k),
                       hT[(head, k)][:, h * NH : (h + 1) * NH],
                       k == 0, k == 1)

            # ---- k0 pass: stream behind the c1/c2 chunk DMAs ------------
            for head, m in [("lv", 0), ("lv", 1), ("mu", 0), ("mu", 1)]:
                l1_mm(head, m, 0, 0)
            for head, m in [("lv", 0), ("lv", 1), ("mu", 0), ("mu", 1)]:
                l1_mm(head, m, 0, 1)
            # ---- k1 pass: lv groups first so L2 lv (and its copy) is early
            l1_mm("lv", 0, 1, 0)
            l1_mm("lv", 1, 1, 0)
            l1_mm("lv", 0, 1, 1)
            l1_mm("lv", 1, 1, 1)
            relu_half("lv", 0, 0)
            relu_half("lv", 1, 0)
            relu_half("lv", 0, 1)
            relu_half("lv", 1, 1)
            l1_mm("mu", 0, 1, 0)
            l1_mm("mu", 1, 1, 0)
            l1_mm("mu", 0, 1, 1)
            l1_mm("mu", 1, 1, 1)
            relu_half("mu", 0, 0)
            relu_half("mu", 0, 1)
            relu_half("mu", 1, 0)
            relu_half("mu", 1, 1)
            # ---- L2 -----------------------------------------------------
            l2_half("lv", 0)
            l2_half("lv", 1)
            l2_half("mu", 0)
            l2_half("mu", 1)
            lv_ps = ps_all[:, L2_BANK["lv"] : L2_BANK["lv"] + NH]
            mu_ps = ps_all[:, L2_BANK["mu"] : L2_BANK["mu"] + NH]

            # ---- tail: PSUM -> SBUF copies + DMA out --------------------
            lv_sb = wk.tile([128, NH], f32, tag="lv_sb")
            mu_sb = wk.tile([128, NH], f32, tag="mu_sb")
            i = nc.scalar.activation(out=lv_sb, in_=lv_ps, func=AF.Copy)
            chain("act", i)
            nc.sync.dma_start(out=olv, in_=lv_sb)
            i = nc.scalar.activation(out=mu_sb, in_=mu_ps, func=AF.Copy)
            chain("act", i)
            nc.sync.dma_start(out=omu, in_=mu_sb)

    nc.compile()
    return nc


def _get_nc():
    if "nc" not in _CACHE:
        _CACHE["nc"] = _build_nc()
    return _CACHE["nc"]


def _make_in_maps(inputs):
    import ml_dtypes

    bf16 = ml_dtypes.bfloat16
    # convert everything to numpy up front: slicing jax arrays here could
    # otherwise dispatch to the (axon) device backend
    emb_x = np.asarray(inputs["emb_x"], dtype=np.float32)

    mu_w1 = np.asarray(inputs["mu_w1"], np.float32)
    lv_w1 = np.asarray(inputs["lv_w1"], np.float32)
    mu_w2 = np.asarray(inputs["mu_w2"], np.float32)
    lv_w2 = np.asarray(inputs["lv_w2"], np.float32)

    bias = np.zeros((128, 4), dtype=np.float32)
    bias[:, 0] = np.asarray(inputs["mu_b1"][:128], np.float32)
    bias[:, 1] = np.asarray(inputs["mu_b1"][128:], np.float32)
    bias[:, 2] = np.asarray(inputs["lv_b1"][:128], np.float32)
    bias[:, 3] = np.asarray(inputs["lv_b1"][128:], np.float32)

    w2pack = np.concatenate(
        [mu_w2[0:128], lv_w2[0:128], mu_w2[128:256], lv_w2[128:256]], axis=1
    )  # (128, 256)

    in_maps = []
    for c in range(NCORES):
        rows = slice(c * NLOC, (c + 1) * NLOC)
        xT = emb_x[rows].T  # (256, 1024)
        pk = np.concatenate(
            [
                lv_w1[0:128],
                xT[0:128, 0:NH],
                mu_w1[0:128],
                xT[0:128, NH:NLOC],
                xT[128:256, :],
                mu_w1[128:256],
                lv_w1[128:256],
                w2pack,
            ],
            axis=1,
        )  # (128, 3328)
        in_maps.append(
            {
                "pk": np.ascontiguousarray(pk.astype(bf16)),
                "bias": bias,
            }
        )
    return in_maps


def kernel(emb_x, emb_y, mu_w1, mu_b1, mu_w2, mu_b2, lv_w1, lv_b1, lv_w2, lv_b2):
    from concourse.bass_utils import run_bass_kernel_spmd

    emb_y = np.asarray(emb_y, dtype=np.float32)
    in_maps = _make_in_maps(
        {
            "emb_x": emb_x,
            "mu_w1": mu_w1,
            "mu_b1": mu_b1,
            "mu_w2": mu_w2,
            "lv_w1": lv_w1,
            "lv_b1": lv_b1,
            "lv_w2": lv_w2,
        }
    )

    nc = _get_nc()
    res = run_bass_kernel_spmd(nc, in_maps, list(range(NCORES)))

    b2mu = np.asarray(mu_b2, np.float64)  # (64,)
    b2lv = np.asarray(lv_b2, np.float64)
    B = np.zeros(DY)
    E = np.zeros(DY)
    A = 0.0
    C = 0.0
    for c in range(NCORES):
        yT = emb_y[c * NLOC : (c + 1) * NLOC].T.astype(np.float64)  # (64,1024)
        mu_st = res.results[c]["omu"].astype(np.float64)  # (128, 512)
        lv_st = res.results[c]["olv"].astype(np.float64)
        # unstack: partitions 0:64 = n cols 0:512, 64:128 = cols 512:1024
        mu = np.concatenate([mu_st[0:DY], mu_st[DY:]], axis=1) + b2mu[:, None]
        lv_raw = np.concatenate([lv_st[0:DY], lv_st[DY:]], axis=1)
        ivc = np.exp(-np.tanh(lv_raw + b2lv[:, None]))
        mic = mu * ivc
        B += ivc.sum(axis=1)
        E += mic.sum(axis=1)
        A += (ivc * yT**2).sum()
        C += (mic * yT).sum()

    y64 = emb_y.astype(np.float64)
    ybar = y64.mean(axis=0)
    y2bar = (y64**2).mean(axis=0)

    total = A - 2.0 * C + (2.0 * E * ybar - B * y2bar).sum()
    loss = -0.5 / N * total
    return np.float32(loss)
